# revision 11
# baseline (speedup 1.0000x reference)
"""CrossMLA Trainium2 kernel: 8-core SPMD, two launches.

Layout convention: activations are feature-major ("transposed", [feat, tok])
on device; the host transposes inputs/outputs. Matmuls run as float32r
(full-rate fp32 mode); every on-device matmul operand is materialized by an
f32r-writing instruction so the rounding chain verifies.

Launch 1 (token-sharded): core (b, g) handles batch b, token slice g*512..:
  down-projections, layernorms, K-rope for its slice.
Launch 2 (head-sharded): core (b, g) handles batch b, heads 4g..4g+3:
  up-projections, attention, partial output projection. Host sums the 4
  partials per batch and transposes back.
"""

import sys

sys.path.insert(0, "/opt/trn_rl_repo")

from contextlib import ExitStack

import numpy as np

import concourse.bass as bass  # noqa: F401
import concourse.tile as tile
from concourse import bacc, mybir
from concourse.bass_utils import run_bass_kernel_spmd

F32 = mybir.dt.float32
F32R = mybir.dt.float32r
BF16 = mybir.dt.bfloat16
AF = mybir.ActivationFunctionType

D = 2048          # d_model
H = 16            # n_heads
DH = 128          # head dim
QP = 1024         # q lora rank
KVP = 1365        # kv lora rank
RP = 64           # rope dims
NOPE = 64
KVTOT = KVP + RP  # 1429
B = 2
S = 2048          # Sq == Skv
TOK = S // 4      # 512, launch-1 token slice
EPS = 1e-5
SCALE = 1.0 / np.sqrt(128.0)
KVC = 11          # ceil(1365/128) LN chunks (last has 85 rows)
KVPAD = KVC * 128  # 1408


# ---------------------------------------------------------------- launch 1
def build_launch1():
    nc = bacc.Bacc("TRN2", target_bir_lowering=False, debug=False, num_devices=8)
    xq = nc.dram_tensor("xq", [D, TOK], F32R, kind="ExternalInput").ap()
    xkv = nc.dram_tensor("xkv", [D, TOK], F32R, kind="ExternalInput").ap()
    wdq = nc.dram_tensor("wdq", [D, QP], F32R, kind="ExternalInput").ap()
    wdkv = nc.dram_tensor("wdkv", [D, KVTOT], F32R, kind="ExternalInput").ap()
    cosk = nc.dram_tensor("cosk", [RP, TOK], F32, kind="ExternalInput").ap()
    sink = nc.dram_tensor("sink", [RP, TOK], F32, kind="ExternalInput").ap()
    cqn = nc.dram_tensor("cqn", [QP, TOK], F32, kind="ExternalOutput").ap()
    ckvn = nc.dram_tensor("ckvn", [KVP, TOK], F32, kind="ExternalOutput").ap()
    kroped = nc.dram_tensor("kroped", [RP, TOK], F32, kind="ExternalOutput").ap()
    ckvraw = nc.dram_tensor("ckvraw", [KVTOT, TOK], F32, kind="ExternalOutput").ap()

    wdq_r = wdq.rearrange("(c p) q -> p c q", p=128)     # [128, 16, 1024]
    wdkv_r = wdkv.rearrange("(c p) q -> p c q", p=128)   # [128, 16, 1429]

    with tile.TileContext(nc) as tc, ExitStack() as ctx:
        xp = ctx.enter_context(tc.tile_pool(name="xp", bufs=1))
        wp = ctx.enter_context(tc.tile_pool(name="wp", bufs=3))
        rawp = ctx.enter_context(tc.tile_pool(name="rawp", bufs=1))
        sqp = ctx.enter_context(tc.tile_pool(name="sqp", bufs=3))
        outp = ctx.enter_context(tc.tile_pool(name="outp", bufs=3))
        statp = ctx.enter_context(tc.tile_pool(name="statp", bufs=2))
        constp = ctx.enter_context(tc.tile_pool(name="constp", bufs=1))
        psa = ctx.enter_context(tc.tile_pool(name="psa", bufs=2, space="PSUM"))
        pss = ctx.enter_context(tc.tile_pool(name="pss", bufs=1, space="PSUM"))
        psb = ctx.enter_context(tc.tile_pool(name="psb", bufs=1, space="PSUM"))

        xq_sb = xp.tile([128, 16, TOK], F32R)
        nc.sync.dma_start(out=xq_sb, in_=xq.rearrange("(c p) t -> p c t", p=128))
        xkv_sb = xp.tile([128, 16, TOK], F32R)
        nc.sync.dma_start(out=xkv_sb, in_=xkv.rearrange("(c p) t -> p c t", p=128))

        ones_f = constp.tile([128, 128], F32, name="ones_f")
        nc.vector.memset(ones_f, 1.0)
        ones = constp.tile([128, 128], F32R, name="ones")
        nc.vector.tensor_copy(ones, ones_f)
        epst = constp.tile([1, 1], F32)
        nc.vector.memset(epst, EPS)

        def ln_block(n_chunks, n_feat, w_r, x_sb, raw_sb, out_dram, raw_dram):
            """down-proj chunks, LN over n_feat rows; also exact raw output."""
            sum_ps = pss.tile([1, TOK], F32, name="sum_ps")
            ssq_ps = pss.tile([1, TOK], F32, name="ssq_ps")
            for m in range(n_chunks):
                rows = min(128, n_feat - m * 128)
                wm = wp.tile([128, 16, 128], F32R, tag="wm", name="wm")
                nc.sync.dma_start(
                    out=wm[:, :, :rows], in_=w_r[:, :, m * 128 : m * 128 + rows]
                )
                ps = psa.tile([128, TOK], F32, tag="dps", name="dps")
                for kc in range(16):
                    nc.tensor.matmul(
                        ps[:rows, :], wm[:, kc, :rows], x_sb[:, kc, :],
                        start=(kc == 0), stop=(kc == 15),
                    )
                nc.vector.tensor_copy(raw_sb[:rows, m, :], ps[:rows, :])
                if raw_dram is not None:
                    st = outp.tile([128, TOK], F32, tag="rawst", name="rawst")
                    nc.scalar.copy(st[:rows, :], ps[:rows, :])
                    nc.sync.dma_start(
                        out=raw_dram[m * 128 : m * 128 + rows, :], in_=st[:rows, :]
                    )
                sq = sqp.tile([128, TOK], F32R, tag="sq", name="sq")
                nc.scalar.activation(sq[:rows, :], ps[:rows, :], AF.Square)
                nc.tensor.matmul(
                    sum_ps, ones[:rows, 0:1], raw_sb[:rows, m, :],
                    start=(m == 0), stop=(m == n_chunks - 1),
                )
                nc.tensor.matmul(
                    ssq_ps, ones[:rows, 0:1], sq[:rows, :],
                    start=(m == 0), stop=(m == n_chunks - 1),
                )
            mean = statp.tile([1, TOK], F32, tag="mean", name="mean")
            nc.scalar.mul(mean, sum_ps, 1.0 / n_feat)
            ex2 = statp.tile([1, TOK], F32, tag="ex2", name="ex2")
            nc.scalar.mul(ex2, ssq_ps, 1.0 / n_feat)
            var = statp.tile([1, TOK], F32, tag="var", name="var")
            nc.vector.tensor_mul(var, mean, mean)
            nc.vector.tensor_sub(var, ex2, var)
            std = statp.tile([1, TOK], F32, tag="std", name="std")
            nc.scalar.activation(std, var, AF.Sqrt, bias=epst)
            rstd = statp.tile([1, TOK], F32, tag="rstd", name="rstd")
            nc.vector.reciprocal(rstd, std)
            mean_r = statp.tile([1, TOK], F32R, tag="mean_r", name="mean_r")
            nc.vector.tensor_copy(mean_r, mean)
            rstd_r = statp.tile([1, TOK], F32R, tag="rstd_r", name="rstd_r")
            nc.vector.tensor_copy(rstd_r, rstd)
            mean_b = psb.tile([128, TOK], F32, tag="meanb", name="meanb")
            nc.tensor.matmul(mean_b, ones[0:1, :], mean_r, start=True, stop=True)
            rstd_b = psb.tile([128, TOK], F32, tag="rstdb", name="rstdb")
            nc.tensor.matmul(rstd_b, ones[0:1, :], rstd_r, start=True, stop=True)
            for m in range(n_chunks):
                rows = min(128, n_feat - m * 128)
                t = outp.tile([128, TOK], F32, tag="lnout", name="lnout")
                nc.vector.tensor_sub(
                    t[:rows, :], raw_sb[:rows, m, :], mean_b[:rows, :]
                )
                nc.vector.tensor_mul(t[:rows, :], t[:rows, :], rstd_b[:rows, :])
                nc.sync.dma_start(
                    out=out_dram[m * 128 : m * 128 + rows, :], in_=t[:rows, :]
                )

        # ---- Q path
        cq_raw = rawp.tile([128, 8, TOK], F32R)
        ln_block(8, QP, wdq_r, xq_sb, cq_raw, cqn, None)

        # ---- KV path (LN part) + exact raw output
        ckv_raw = rawp.tile([128, 12, TOK], F32R)
        ln_block(KVC, KVP, wdkv_r, xkv_sb, ckv_raw, ckvn, ckvraw)

        # ---- krope chunk: features 1365..1429 of ckv
        wm = wp.tile([128, 16, 128], F32R, tag="wm")
        nc.sync.dma_start(out=wm[:, :, :RP], in_=wdkv_r[:, :, KVP:KVTOT])
        ps = psa.tile([128, TOK], F32, tag="dps")
        for kc in range(16):
            nc.tensor.matmul(
                ps[:RP, :], wm[:, kc, :RP], xkv_sb[:, kc, :],
                start=(kc == 0), stop=(kc == 15),
            )
        kr_raw = ckv_raw[0:RP, 11, :]
        nc.vector.tensor_copy(kr_raw, ps[:RP, :])
        st = outp.tile([128, TOK], F32, tag="rawst")
        nc.scalar.copy(st[:RP, :], ps[:RP, :])
        nc.sync.dma_start(out=ckvraw[KVP:KVTOT, :], in_=st[:RP, :])
        # rope: out = raw*cos + shift32(raw)*sin_signed
        cos_sb = constp.tile([RP, TOK], F32)
        nc.sync.dma_start(out=cos_sb, in_=cosk)
        sin_sb = constp.tile([RP, TOK], F32)
        nc.sync.dma_start(out=sin_sb, in_=sink)
        shift = outp.tile([RP, TOK], F32R, tag="krshift")
        nc.sync.dma_start(out=shift[0:32, :], in_=kr_raw[32:64, :])
        nc.sync.dma_start(out=shift[32:64, :], in_=kr_raw[0:32, :])
        t1 = outp.tile([RP, TOK], F32, tag="krt1")
        nc.vector.tensor_mul(t1, kr_raw, cos_sb)
        t2 = outp.tile([RP, TOK], F32, tag="krt2")
        nc.vector.tensor_mul(t2, shift, sin_sb)
        nc.vector.tensor_add(t1, t1, t2)
        nc.sync.dma_start(out=kroped, in_=t1)

    nc.compile()
    return nc


# ---------------------------------------------------------------- launch 2
def build_launch2():
    nc = bacc.Bacc("TRN2", target_bir_lowering=False, debug=False, num_devices=8)
    cqn = nc.dram_tensor("cqn", [QP, S], F32R, kind="ExternalInput").ap()
    ckvn = nc.dram_tensor("ckvn", [KVPAD, S], F32R, kind="ExternalInput").ap()
    krop = nc.dram_tensor("krop", [RP, S], F32, kind="ExternalInput").ap()
    wuq = nc.dram_tensor("wuq", [QP, 512], F32R, kind="ExternalInput").ap()
    wuk = nc.dram_tensor("wuk", [KVPAD, 256], F32R, kind="ExternalInput").ap()
    wuv = nc.dram_tensor("wuv", [KVPAD, 512], F32R, kind="ExternalInput").ap()
    wo = nc.dram_tensor("wo", [512, D], F32R, kind="ExternalInput").ap()
    cosq = nc.dram_tensor("cosq", [128, S], F32, kind="ExternalInput").ap()
    sinq = nc.dram_tensor("sinq", [128, S], F32, kind="ExternalInput").ap()
    yt = nc.dram_tensor("yt", [D, S], F32, kind="ExternalOutput").ap()

    cqn_r = cqn.rearrange("(c p) t -> p c t", p=128)    # [128, 8, S]
    ckvn_r = ckvn.rearrange("(c p) t -> p c t", p=128)  # [128, 11, S]
    wuq_r = wuq.rearrange("(c p) q -> p c q", p=128)    # [128, 8, 512]
    wuk_r = wuk.rearrange("(c p) q -> p c q", p=128)    # [128, 11, 256]
    wuv_r = wuv.rearrange("(c p) q -> p c q", p=128)    # [128, 11, 512]
    wo_r = wo.rearrange("(c p) q -> p c q", p=128)      # [128, 4, 2048]
    yt_r = yt.rearrange("(c p) t -> p c t", p=128)      # [128, 16, S]

    osc = nc.dram_tensor("osc", [4, 128, S], F32R).ap()  # attention-out scratch
    osc_r = osc.rearrange("h p t -> p h t")

    with tile.TileContext(nc) as tc, ExitStack() as ctx:
        pp = ctx.enter_context(tc.tile_pool(name="pp", bufs=4))
        accp = ctx.enter_context(tc.tile_pool(name="accp", bufs=2))
        smallp = ctx.enter_context(tc.tile_pool(name="smallp", bufs=3))
        outp = ctx.enter_context(tc.tile_pool(name="outp", bufs=3))
        constp = ctx.enter_context(tc.tile_pool(name="constp", bufs=1))
        psa = ctx.enter_context(tc.tile_pool(name="psa", bufs=2, space="PSUM"))
        psl = ctx.enter_context(tc.tile_pool(name="psl", bufs=2, space="PSUM"))
        psv = ctx.enter_context(tc.tile_pool(name="psv", bufs=2, space="PSUM"))
        pssm = ctx.enter_context(tc.tile_pool(name="pssm", bufs=1, space="PSUM"))

        ones_f = constp.tile([128, 128], F32, name="ones_f")
        nc.vector.memset(ones_f, 1.0)
        ones = constp.tile([128, 128], F32R, name="ones")
        nc.vector.tensor_copy(ones, ones_f)
        cos_sb = constp.tile([128, S], F32)
        nc.sync.dma_start(out=cos_sb, in_=cosq)
        sin_sb = constp.tile([128, S], F32)
        nc.sync.dma_start(out=sin_sb, in_=sinq)
        # krope duplicated into both partition halves
        krope_sb = constp.tile([128, S], F32)
        nc.sync.dma_start(out=krope_sb[0:64, :], in_=krop)
        nc.sync.dma_start(out=krope_sb[64:128, :], in_=krop)

        with tc.tile_pool(name="bigp", bufs=1) as bigp:
            # per-head tiles (resident through attention)
            q_t = [bigp.tile([128, S], F32R, tag=f"q{h}", name=f"q{h}")
                   for h in range(4)]
            k_t = [bigp.tile([128, S], F32R, tag=f"k{h}", name=f"k{h}")
                   for h in range(4)]
            v_t = [bigp.tile([128, 4, 128], BF16, tag=f"v{t}", name=f"v{t}")
                   for t in range(16)]

            # ---- Q up-projection (+rope), streamed per 512-token chunk
            with tc.tile_pool(name="qph", bufs=1) as qph:
                wuq_sb = qph.tile([128, 8, 512], F32R)
                nc.sync.dma_start(out=wuq_sb, in_=wuq_r)
                for qc in range(4):
                    blk = qph.tile([128, 8, 512], F32R, tag="cqblk", bufs=1,
                                   name="cqblk")
                    nc.sync.dma_start(
                        out=blk, in_=cqn_r[:, :, qc * 512 : (qc + 1) * 512])
                    for h in range(4):
                        ps = psa.tile([128, 512], F32, tag="acc", name="qps")
                        for kc in range(8):
                            nc.tensor.matmul(
                                ps, wuq_sb[:, kc, h * 128 : (h + 1) * 128],
                                blk[:, kc, :], start=(kc == 0), stop=(kc == 7),
                            )
                        nc.vector.tensor_copy(
                            q_t[h][:, qc * 512 : (qc + 1) * 512], ps)
                # rope on q (even head: rope rows 64..127; odd: rows 0..63)
                for h in range(4):
                    rb = 64 if h % 2 == 0 else 0
                    qr = q_t[h][rb : rb + 64, :]
                    sh = qph.tile([128, S], F32R, tag="qshift", bufs=1, name="sh")
                    nc.sync.dma_start(out=sh[rb : rb + 32, :], in_=qr[32:64, :])
                    nc.sync.dma_start(out=sh[rb + 32 : rb + 64, :], in_=qr[0:32, :])
                    t1 = qph.tile([128, S], F32, tag="qt1", bufs=1, name="t1")
                    nc.vector.tensor_mul(
                        t1[rb : rb + 64, :], qr, cos_sb[rb : rb + 64, :])
                    t2 = qph.tile([128, S], F32, tag="qt2", bufs=1, name="t2")
                    nc.vector.tensor_mul(
                        t2[rb : rb + 64, :], sh[rb : rb + 64, :],
                        sin_sb[rb : rb + 64, :])
                    nc.vector.tensor_add(
                        qr, t1[rb : rb + 64, :], t2[rb : rb + 64, :])

            # ---- K/V up-projection, streamed per 512-token chunk
            with tc.tile_pool(name="kvph", bufs=1) as kvph:
                wuk_sb = kvph.tile([128, KVC, 256], F32R)
                nc.sync.dma_start(out=wuk_sb, in_=wuk_r)
                wuv_sb = kvph.tile([128, KVC, 512], F32R)
                nc.sync.dma_start(out=wuv_sb, in_=wuv_r)
                for tc4 in range(4):
                    blk = kvph.tile([128, KVC, 512], F32R, tag="kvblk", bufs=1,
                                    name="kvblk")
                    nc.sync.dma_start(
                        out=blk, in_=ckvn_r[:, :, tc4 * 512 : (tc4 + 1) * 512])
                    for pr in range(2):  # head pairs (0,1) and (2,3)
                        ps = psa.tile([128, 512], F32, tag="acc", name="kps")
                        for kc in range(KVC):
                            nc.tensor.matmul(
                                ps, wuk_sb[:, kc, pr * 128 : (pr + 1) * 128],
                                blk[:, kc, :],
                                start=(kc == 0), stop=(kc == KVC - 1),
                            )
                        # even head: nope rows 0..63; odd head: rows 64..127
                        sl = slice(tc4 * 512, (tc4 + 1) * 512)
                        nc.vector.tensor_copy(k_t[2 * pr][0:64, sl], ps[0:64, :])
                        nc.vector.tensor_copy(
                            k_t[2 * pr + 1][64:128, sl], ps[64:128, :])
                    for tci in range(4):
                        ti = tc4 * 4 + tci
                        ps = psv.tile([128, 512], F32, tag="v", name="vps")
                        for kc in range(KVC):
                            nc.tensor.matmul(
                                ps, blk[:, kc, tci * 128 : (tci + 1) * 128],
                                wuv_sb[:, kc, :],
                                start=(kc == 0), stop=(kc == KVC - 1),
                            )
                        nc.vector.tensor_copy(
                            v_t[ti].rearrange("p h d -> p (h d)"), ps)
            # krope halves into K tiles
            for h in range(4):
                rb = 64 if h % 2 == 0 else 0
                nc.vector.tensor_copy(k_t[h][rb : rb + 64, :],
                                      krope_sb[rb : rb + 64, :])

            # ---- attention
            for h in range(4):
                for qc in range(4):
                    qsl = slice(qc * 512, (qc + 1) * 512)
                    pv = psv.tile([128, 512], F32, tag="v", name="pvps")
                    pacc = accp.tile([128, 512], F32, tag="pacc", name="pacc")
                    for k in range(16):
                        lg = psl.tile([128, 512], F32, tag="lgps", name="lgps")
                        nc.tensor.matmul(
                            lg, k_t[h][:, k * 128 : (k + 1) * 128],
                            q_t[h][:, qsl], start=True, stop=True,
                        )
                        pk = pp.tile([128, 512], BF16, tag="pk", name="pk")
                        nc.scalar.activation(pk, lg, AF.Exp, scale=SCALE)
                        nc.tensor.matmul(
                            pv, v_t[k][:, h, :], pk, start=(k == 0), stop=(k == 15)
                        )
                        if k == 0:
                            nc.vector.tensor_copy(pacc, pk)
                        else:
                            nc.vector.tensor_add(pacc, pacc, pk)
                    pacc_r = accp.tile([128, 512], F32R, tag="pacc_r",
                                       name="pacc_r")
                    nc.vector.tensor_copy(pacc_r, pacc)
                    den = pssm.tile([1, 512], F32, tag="den", name="den")
                    nc.tensor.matmul(den, ones[:, 0:1], pacc_r,
                                     start=True, stop=True)
                    dinv = smallp.tile([1, 512], F32, tag="dinv", name="dinv")
                    nc.vector.reciprocal(dinv, den)
                    dinv_r = smallp.tile([1, 512], F32R, tag="dinv_r",
                                         name="dinv_r")
                    nc.vector.tensor_copy(dinv_r, dinv)
                    dinv_b = pssm.tile([128, 512], F32, tag="dinvb", name="dinvb")
                    nc.tensor.matmul(dinv_b, ones[0:1, :], dinv_r,
                                     start=True, stop=True)
                    dinvb_sb = outp.tile([128, 512], F32, tag="dinvb_sb",
                                          name="dinvb_sb")
                    nc.scalar.copy(dinvb_sb, dinv_b)
                    ot = outp.tile([128, 512], F32R, tag="oout", name="oout")
                    nc.vector.tensor_mul(ot, pv, dinvb_sb)
                    nc.sync.dma_start(out=osc_r[:, h, qsl], in_=ot)

        # ---- output projection (partial over this core's 4 head-dim chunks)
        with tc.tile_pool(name="oph", bufs=1) as oph:
            wo_sb = oph.tile([128, 4, D], F32R)
            nc.sync.dma_start(out=wo_sb, in_=wo_r)
            for qc in range(4):
                qsl = slice(qc * 512, (qc + 1) * 512)
                osb = oph.tile([128, 4, 512], F32R, tag="osb", bufs=2, name="osb")
                nc.sync.dma_start(out=osb, in_=osc_r[:, :, qsl])
                for of in range(16):
                    ps = psa.tile([128, 512], F32, tag="acc", name="ops")
                    for hd in range(4):
                        nc.tensor.matmul(
                            ps, wo_sb[:, hd, of * 128 : (of + 1) * 128],
                            osb[:, hd, :], start=(hd == 0), stop=(hd == 3),
                        )
                    ot = outp.tile([128, 512], F32, tag="yout", name="yout")
                    nc.vector.tensor_copy(ot, ps)
                    nc.sync.dma_start(out=yt_r[:, of, qsl], in_=ot)

    nc.compile()
    return nc


# ---------------------------------------------------------------- host glue
_NC_CACHE = {}


def _get_ncs():
    if "nc1" not in _NC_CACHE:
        _NC_CACHE["nc1"] = build_launch1()
        _NC_CACHE["nc2"] = build_launch2()
    return _NC_CACHE["nc1"], _NC_CACHE["nc2"]


def _rope_tables(cos_cached, sin_cached):
    """feature-major [64, S] interleave-repeated cos / sign-folded sin."""
    cos = np.asarray(cos_cached, np.float32)[:S, : RP // 2]  # [S, 32]
    sin = np.asarray(sin_cached, np.float32)[:S, : RP // 2]
    cos_rep = np.repeat(cos, 2, axis=1).T.copy()             # [64, S]
    sin_rep = np.repeat(sin, 2, axis=1).T.copy()
    sin_signed = sin_rep.copy()
    sin_signed[0:32] *= -1.0
    return cos_rep, sin_signed


def kernel(q_input, kv_input, W_dq, W_uq, W_dkv, W_ukv, W_o,
           cos_cached, sin_cached):
    q_input = np.asarray(q_input, np.float32)
    kv_input = np.asarray(kv_input, np.float32)
    W_dq = np.asarray(W_dq, np.float32)
    W_uq = np.asarray(W_uq, np.float32)
    W_dkv = np.asarray(W_dkv, np.float32)
    W_ukv = np.asarray(W_ukv, np.float32)
    W_o = np.asarray(W_o, np.float32)
    nc1, nc2 = _get_ncs()

    cos_rep, sin_signed = _rope_tables(cos_cached, sin_cached)

    # ---------------- launch 1
    qT = [np.ascontiguousarray(q_input[b].T) for b in range(B)]    # [D, S]
    kvT = [np.ascontiguousarray(kv_input[b].T) for b in range(B)]
    in1 = []
    for c in range(8):
        b, g = c // 4, c % 4
        sl = slice(g * TOK, (g + 1) * TOK)
        in1.append({
            "xq": np.ascontiguousarray(qT[b][:, sl]),
            "xkv": np.ascontiguousarray(kvT[b][:, sl]),
            "wdq": W_dq,
            "wdkv": W_dkv,
            "cosk": np.ascontiguousarray(cos_rep[:, sl]),
            "sink": np.ascontiguousarray(sin_signed[:, sl]),
        })
    r1 = run_bass_kernel_spmd(nc1, in1, list(range(8))).results

    # assemble full-length latents per batch
    cqn_f = [np.concatenate([r1[b * 4 + g]["cqn"] for g in range(4)], axis=1)
             for b in range(B)]                                    # [QP, S]
    ckvn_f = [np.concatenate([r1[b * 4 + g]["ckvn"] for g in range(4)], axis=1)
              for b in range(B)]                                   # [KVP, S]
    krop_f = [np.concatenate([r1[b * 4 + g]["kroped"] for g in range(4)], axis=1)
              for b in range(B)]                                   # [RP, S]
    ckvraw_f = [np.concatenate([r1[b * 4 + g]["ckvraw"] for g in range(4)], axis=1)
                for b in range(B)]                                 # [KVTOT, S]
    compressed_kv = np.stack([a.T for a in ckvraw_f])              # [B, S, KVTOT]

    # pad kv latents' contraction dim to 1408
    ckvn_p = [np.concatenate(
        [a, np.zeros((KVPAD - KVP, S), np.float32)], axis=0) for a in ckvn_f]

    # per-core weight slices (parity-permuted for odd heads)
    cos128 = np.concatenate([cos_rep, cos_rep], axis=0)            # [128, S]
    sin128 = np.concatenate([sin_signed, sin_signed], axis=0)
    wu_resh = W_ukv.reshape(KVP, H, DH + NOPE)
    in2 = []
    for c in range(8):
        b, g = c // 4, c % 4
        heads = [4 * g + j for j in range(4)]
        wuq_cols = []
        for j, h in enumerate(heads):
            cols = W_uq[:, h * DH : (h + 1) * DH]
            if j % 2 == 1:  # odd local head: [rope, nope] layout
                cols = np.concatenate([cols[:, 64:], cols[:, :64]], axis=1)
            wuq_cols.append(cols)
        wuq_c = np.ascontiguousarray(np.concatenate(wuq_cols, axis=1))  # [QP,512]
        wuk_c = np.concatenate([wu_resh[:, h, :NOPE] for h in heads], axis=1)
        wuk_c = np.concatenate(
            [wuk_c, np.zeros((KVPAD - KVP, 256), np.float32)], axis=0)
        wuv_c = np.concatenate([wu_resh[:, h, NOPE:] for h in heads], axis=1)
        wuv_c = np.concatenate(
            [wuv_c, np.zeros((KVPAD - KVP, 512), np.float32)], axis=0)
        wo_c = np.ascontiguousarray(
            W_o.T[heads[0] * DH : (heads[-1] + 1) * DH, :])             # [512, D]
        in2.append({
            "cqn": cqn_f[b],
            "ckvn": ckvn_p[b],
            "krop": krop_f[b],
            "wuq": wuq_c,
            "wuk": np.ascontiguousarray(wuk_c),
            "wuv": np.ascontiguousarray(wuv_c),
            "wo": wo_c,
            "cosq": cos128,
            "sinq": sin128,
        })
    r2 = run_bass_kernel_spmd(nc2, in2, list(range(8))).results

    out = np.stack([
        sum(r2[b * 4 + g]["yt"] for g in range(4)).T for b in range(B)
    ]).astype(np.float32)                                          # [B, S, D]
    return out, compressed_kv
